# revision 6
# baseline (speedup 1.0000x reference)
"""Trainium2 Bass kernel for pointer-generator additive attention.

Full op (per batch b):
    dec_fea = s_t_hat @ W_d.T + b_d                         # (n,)
    att     = EF[b] + dec_fea[None,:] + cov[b][:,None]*W_c  # (t, n)
    score   = tanh(att) @ v                                 # (t,)
    attn    = renorm(softmax(score) * mask)                 # (t,)
    c_t     = attn @ EO[b]                                  # (n,)
    cov_next= cov + attn

Data-parallel over batch across 8 NeuronCores (8 batches/core, params
replicated, no collectives).

v2 design (vs the identity-matmul baseline):
  - EF/EO staged host-side as bf16 in the exact SBUF tile layout
    [119 partitions, 9 chunks x 1024] (t = c*119 + p, zero-padded): one
    fully-contiguous DMA per batch per tensor, half the HBM bytes.
  - T is tiled in chunks of 119 so K = 119 (identity band) + 8 (one-hot
    dec-row selectors) + 1 (cov -> W_c) = 128: a SINGLE matmul per
    (chunk, n-half) computes EF + dec_fea + cov*W_c fused.  The lhsT
    matrix [128, 8*1071] (identity + one-hots + cov row) is prebuilt on
    the host; dec_fea rows + W_c live at partitions 119..127 of the EF
    stream buffers (written once per buffer after the dec matvec).
  - ScalarE: one tanh per chunk (PSUM f32 -> SBUF bf16).
  - VectorE: scalar_tensor_tensor (th * v_bcast) with accum_out gives
    the n-reduction (score) per chunk.
  - Masked softmax + renorm + coverage batched in GROUPS OF 4 on
    [4, 1024] row tiles (one pass instead of per-batch row ops).
  - c_t: per chunk, lhsT = [119, 8] one-hot-column attn (column b =
    attn chunk, rest zero) so all 8 batches accumulate into a single
    [8, 512] x2 PSUM group; one ScalarE copy + one DMA at the end.
All heavy matmuls are bf16 (tolerance 2e-2, measured rel_err ~2.6e-3).
"""

import sys

if "/opt/trn_rl_repo" not in sys.path:
    sys.path.insert(0, "/opt/trn_rl_repo")

import ml_dtypes
import numpy as np

import concourse.bass as bass
import concourse.mybir as mybir
import concourse.tile as tile
from concourse import bacc
from concourse.bass_utils import run_bass_kernel_spmd
from concourse.masks import make_identity

F32 = mybir.dt.float32
BF16 = mybir.dt.bfloat16
AF = mybir.ActivationFunctionType
ALU = mybir.AluOpType

N_CORES = 8
B = 64
NB = B // N_CORES  # local batches per core
T = 1024
N = 1024
CH = 119           # t-chunk height (identity rows in the fused matmul)
NCH = 9            # chunks per batch: 8*119 + 72
LAST = T - (NCH - 1) * CH  # 72
W = NCH * CH       # 1071: per-batch window stride in lhsT
KT = N // 128      # k-tiles for the W_d matvec
GRP = 4            # softmax group size


def build_bass(nb: int = NB) -> bass.Bass:
    nc = bacc.Bacc()

    ef_d = nc.declare_dram_parameter("ef_tiles", [nb, CH, NCH * N], BF16, isOutput=False)
    eo_d = nc.declare_dram_parameter("eo_tiles", [nb, CH, NCH * N], BF16, isOutput=False)
    lhs_d = nc.declare_dram_parameter("lhsT_full", [128, nb * W], BF16, isOutput=False)
    mk_d = nc.declare_dram_parameter("enc_padding_mask", [nb, T], F32, isOutput=False)
    cv_d = nc.declare_dram_parameter("coverage", [nb, T], F32, isOutput=False)
    wdt_d = nc.declare_dram_parameter("W_d_T", [N, N], BF16, isOutput=False)
    st_d = nc.declare_dram_parameter("s_t_hat_T", [N, nb], BF16, isOutput=False)
    bd_d = nc.declare_dram_parameter("b_d", [N], BF16, isOutput=False)
    wc_d = nc.declare_dram_parameter("W_c", [N], BF16, isOutput=False)
    v_d = nc.declare_dram_parameter("v", [N], BF16, isOutput=False)
    ct_o = nc.declare_dram_parameter("c_t", [nb, N], F32, isOutput=True)
    at_o = nc.declare_dram_parameter("attn", [nb, T], F32, isOutput=True)
    cn_o = nc.declare_dram_parameter("coverage_next", [nb, T], F32, isOutput=True)

    with tile.TileContext(nc) as tc:
        with (
            tc.tile_pool(name="consts", bufs=1) as consts,
            tc.tile_pool(name="lhsp", bufs=1) as lhsp,
            tc.tile_pool(name="wdtp", bufs=1) as wdtp,
            tc.tile_pool(name="efp", bufs=2) as efp,
            tc.tile_pool(name="eop", bufs=3) as eop,
            tc.tile_pool(name="thp", bufs=3) as thp,
            tc.tile_pool(name="ttro", bufs=2) as ttro,
            tc.tile_pool(name="smal", bufs=2) as smal,
            tc.tile_pool(name="a9p", bufs=2) as a9p,
            tc.tile_pool(name="acwp", bufs=2) as acwp,
            tc.tile_pool(name="psA", bufs=2, space="PSUM") as psA,
            tc.tile_pool(name="psS", bufs=2, space="PSUM") as psS,
            tc.tile_pool(name="psT", bufs=2, space="PSUM") as psT,
        ):
            # ---------------- constants / small inputs ----------------
            ident = consts.tile([CH, CH], F32)
            make_identity(nc, ident)

            bd_b = consts.tile([1, N], BF16)
            nc.sync.dma_start(out=bd_b, in_=bd_d[None, :])
            v_b = consts.tile([1, N], BF16)
            nc.sync.dma_start(out=v_b, in_=v_d[None, :])
            wc_b = consts.tile([1, N], BF16)
            nc.sync.dma_start(out=wc_b, in_=wc_d[None, :])
            ones8 = consts.tile([1, NB], BF16)
            nc.vector.memset(ones8, 1.0)

            # full lhsT (identity band + one-hot dec selectors + cov row)
            lhs_all = lhsp.tile([128, nb * W], BF16)
            nc.gpsimd.dma_start(out=lhs_all, in_=lhs_d[:, :])

            # per-group f32 rows for softmax/coverage (engine APs must
            # start at a 32-aligned partition, so one [GRP, T] tile per group)
            ngrp = nb // GRP
            mask_g, cov_g, score_g, attn_g, covn_g = [], [], [], [], []
            for g in range(ngrp):
                mg = consts.tile([GRP, T], F32, name=f"mask{g}")
                nc.sync.dma_start(out=mg, in_=mk_d[g * GRP:(g + 1) * GRP, :])
                mask_g.append(mg)
                cg = consts.tile([GRP, T], F32, name=f"cov{g}")
                nc.sync.dma_start(out=cg, in_=cv_d[g * GRP:(g + 1) * GRP, :])
                cov_g.append(cg)
                score_g.append(consts.tile([GRP, T], F32, name=f"score{g}"))
                attn_g.append(consts.tile([GRP, T], F32, name=f"attn{g}"))
                covn_g.append(consts.tile([GRP, T], F32, name=f"covn{g}"))
            ct_sb = consts.tile([nb, N], F32)

            # v broadcast to all partitions for the score reduction
            v_bcast = consts.tile([128, N], BF16)
            nc.gpsimd.partition_broadcast(v_bcast, v_b)

            # W_d.T k-tiles (pre-transposed on the host), one DMA
            wdt_all = wdtp.tile([128, KT, N], BF16)
            nc.sync.dma_start(
                out=wdt_all, in_=wdt_d.rearrange("(kj p) n -> p kj n", p=128)
            )
            sT_all = consts.tile([128, KT, NB], BF16)
            nc.sync.dma_start(
                out=sT_all, in_=st_d.rearrange("(kj p) b -> p kj b", p=128)
            )

            # dec_fea rows = s_t_hat @ W_d.T + b_d   (bf16 matmuls, tiny)
            dec_rows = consts.tile([NB, N], BF16)
            for h in range(2):
                sl = slice(h * 512, (h + 1) * 512)
                psd = psA.tile([NB, 512], F32, tag="att")
                for kj in range(KT):
                    nc.tensor.matmul(
                        psd,
                        lhsT=sT_all[:, kj, :],
                        rhs=wdt_all[:, kj, sl],
                        start=(kj == 0), stop=False,
                    )
                nc.tensor.matmul(
                    psd, lhsT=ones8, rhs=bd_b[0:1, sl],
                    start=False, stop=True,
                )
                nc.scalar.activation(dec_rows[:, sl], psd, AF.Copy)

            # EF stream buffers: partitions 119..126 = dec rows,
            # partition 127 = W_c (constant across batches/chunks)
            ef_bufs = [efp.tile([128, NCH, N], BF16, tag="ef", name=f"efb{i}")
                       for i in range(2)]
            for buf in ef_bufs:
                for c in range(NCH):
                    nc.sync.dma_start(out=buf[119:127, c, :], in_=dec_rows)
                    nc.sync.dma_start(out=buf[127:128, c, :], in_=wc_b)

            # ---------------- pass 1: scores ----------------
            def chunk_m(c):
                return LAST if c == NCH - 1 else CH

            score_cols_t = {}

            def phase_a(b):
                buf = ef_bufs[b % 2]
                nc.gpsimd.dma_start(out=buf[0:CH, :, :], in_=ef_d[b, :, :])

                score_cols = smal.tile([CH, NCH], F32, tag="scol")
                score_cols_t[b] = score_cols
                for c in range(NCH):
                    m = chunk_m(c)
                    att = psA.tile([CH, N], F32, tag="att")
                    for h in range(2):
                        nc.tensor.matmul(
                            att[0:m, h * 512:(h + 1) * 512],
                            lhsT=lhs_all[:, b * W + c * CH: b * W + c * CH + m],
                            rhs=buf[:, c, h * 512:(h + 1) * 512],
                            start=True, stop=True, skip_group_check=True,
                        )
                    th = thp.tile([CH, N], BF16, tag="th")
                    nc.scalar.activation(th[0:m, :], att[0:m, :], AF.Tanh)
                    scr = ttro.tile([CH, N], BF16, tag="ttro")
                    nc.vector.scalar_tensor_tensor(
                        out=scr[0:m, :], in0=th[0:m, :], scalar=1.0,
                        in1=v_bcast[0:m, :],
                        op0=ALU.mult, op1=ALU.mult,
                        accum_out=score_cols[0:m, c:c + 1],
                    )

                # score columns -> row b of score_all (t = c*119 + p)
                ps9 = psT.tile([NCH, CH], F32, tag="tscratch")
                nc.tensor.matmul(
                    ps9, lhsT=score_cols, rhs=ident, is_transpose=True,
                    start=True, stop=True,
                )
                score9 = smal.tile([NCH, CH], F32, tag="s9")
                nc.scalar.activation(score9, ps9, AF.Copy)
                srow = score_g[b // GRP]
                r = b % GRP
                nc.sync.dma_start(
                    out=srow[r:r + 1, 0:(NCH - 1) * CH].rearrange(
                        "p (c w) -> p c w", c=NCH - 1),
                    in_=score9[0:NCH - 1, :],
                )
                nc.sync.dma_start(
                    out=srow[r:r + 1, (NCH - 1) * CH:T],
                    in_=score9[NCH - 1:NCH, 0:LAST],
                )

            # masked softmax + renorm + coverage for GRP batches at once
            def softmax_group(g):
                # scores are O(1) (|s| < ~3): plain exp is safe, skip max-sub
                nc.scalar.activation(attn_g[g], score_g[g], AF.Exp)
                ssum = smal.tile([GRP, 1], F32, tag="ssum")
                nc.vector.scalar_tensor_tensor(
                    out=attn_g[g], in0=attn_g[g], scalar=1.0,
                    in1=mask_g[g],
                    op0=ALU.mult, op1=ALU.mult, accum_out=ssum,
                )
                rs = smal.tile([GRP, 1], F32, tag="rs")
                nc.vector.reciprocal(rs, ssum)
                nc.vector.tensor_scalar_mul(attn_g[g], attn_g[g], rs)
                nc.vector.tensor_add(covn_g[g], cov_g[g], attn_g[g])

            # ---------------- pass 2: context vectors ----------------
            eo_bufs = {}

            def load_eo(b):
                buf = eop.tile([CH, NCH, N], BF16, tag="eo")
                nc.sync.dma_start(out=buf, in_=eo_d[b, :, :])
                eo_bufs[b] = buf

            ctps = [psS.tile([NB, 512], F32, tag="srow", name=f"ctp{h}")
                    for h in range(2)]

            def phase_c(b):
                # attn row -> [9, 119] -> transpose -> one-hot column b
                attn9 = a9p.tile([NCH, CH], F32, tag="attn9")
                arow = attn_g[b // GRP]
                r = b % GRP
                nc.sync.dma_start(
                    out=attn9[0:NCH - 1, :],
                    in_=arow[r:r + 1, 0:(NCH - 1) * CH].rearrange(
                        "p (c w) -> p c w", c=NCH - 1),
                )
                nc.sync.dma_start(
                    out=attn9[NCH - 1:NCH, 0:LAST],
                    in_=arow[r:r + 1, (NCH - 1) * CH:T],
                )
                acp = psT.tile([CH, NCH], F32, tag="tscratch")
                nc.tensor.matmul(
                    acp, lhsT=attn9, rhs=ident[0:NCH, 0:NCH],
                    is_transpose=True, start=True, stop=True,
                )
                acw = acwp.tile([CH, NCH, NB], BF16, tag="acw")
                nc.gpsimd.memset(acw, 0.0)
                nc.scalar.activation(acw[:, :, b], acp, AF.Copy)

                buf = eo_bufs.pop(b)
                for c in range(NCH):
                    m = chunk_m(c)
                    for h in range(2):
                        nc.tensor.matmul(
                            ctps[h],
                            lhsT=acw[0:m, c, :],
                            rhs=buf[0:m, c, h * 512:(h + 1) * 512],
                            start=(b == 0 and c == 0),
                            stop=(b == nb - 1 and c == NCH - 1),
                            skip_group_check=True,
                        )

            # ---------------- schedule ----------------
            # pass-1 front, softmax per half, pass-2 interleaved one group back
            for b in range(GRP):
                phase_a(b)
                if b >= 2:
                    load_eo(b - 2)
            softmax_group(0)
            for b in range(GRP, nb):
                phase_a(b)
                load_eo(b - 2)
                phase_c(b - GRP)
            softmax_group(1)
            load_eo(nb - 2)
            load_eo(nb - 1)
            for b in range(GRP, nb):
                phase_c(b)

            for h in range(2):
                nc.scalar.activation(
                    ct_sb[:, h * 512:(h + 1) * 512], ctps[h], AF.Copy
                )
            nc.sync.dma_start(out=ct_o[:, :], in_=ct_sb)
            for g in range(ngrp):
                rows = slice(g * GRP, (g + 1) * GRP)
                nc.sync.dma_start(out=at_o[rows, :], in_=attn_g[g])
                nc.sync.dma_start(out=cn_o[rows, :], in_=covn_g[g])

    nc.finalize()
    return nc


_CACHE: dict = {}


def _get_nc() -> bass.Bass:
    if "nc" not in _CACHE:
        _CACHE["nc"] = build_bass(NB)
    return _CACHE["nc"]


def _stage_tiles(x):
    """[T, N] f32 -> [119, 9*1024] bf16, t = c*119 + p, zero-padded."""
    pad = np.zeros((CH * NCH, N), np.float32)
    pad[:T] = x
    return (
        pad.reshape(NCH, CH, N).transpose(1, 0, 2).reshape(CH, NCH * N)
    ).astype(ml_dtypes.bfloat16)


def _build_lhs(cov_core):
    """[128, 8*1071] bf16: identity band + one-hot rows + cov row."""
    lhs = np.zeros((128, NB * W), np.float32)
    eye = np.eye(CH, dtype=np.float32)
    lhs[:CH] = np.tile(eye, (1, NB * NCH))
    for j in range(NB):
        lhs[CH + j, j * W:(j + 1) * W] = 1.0
        lhs[127, j * W:j * W + T] = cov_core[j]
    return lhs.astype(ml_dtypes.bfloat16)


def make_in_maps(inputs: dict) -> list:
    f = lambda x: np.ascontiguousarray(np.asarray(x), dtype=np.float32)
    s = f(inputs["s_t_hat"])
    eo = f(inputs["encoder_outputs"])
    ef = f(inputs["encoder_feature"]).reshape(B, T, N)
    mk = f(inputs["enc_padding_mask"])
    cv = f(inputs["coverage"])
    wdt = np.ascontiguousarray(f(inputs["W_d"]).T).astype(ml_dtypes.bfloat16)
    bd = f(inputs["b_d"])
    wc = f(inputs["W_c"])
    vv = f(inputs["v"])
    in_maps = []
    for i in range(N_CORES):
        sl = slice(i * NB, (i + 1) * NB)
        ef_tiles = np.stack([_stage_tiles(ef[i * NB + j]) for j in range(NB)])
        eo_tiles = np.stack([_stage_tiles(eo[i * NB + j]) for j in range(NB)])
        in_maps.append({
            "ef_tiles": ef_tiles,
            "eo_tiles": eo_tiles,
            "lhsT_full": _build_lhs(cv[sl]),
            "s_t_hat_T": np.ascontiguousarray(s[sl].T).astype(ml_dtypes.bfloat16),
            "enc_padding_mask": mk[sl],
            "coverage": cv[sl],
            "W_d_T": wdt,
            "b_d": bd.astype(ml_dtypes.bfloat16),
            "W_c": wc.astype(ml_dtypes.bfloat16),
            "v": vv.astype(ml_dtypes.bfloat16),
        })
    return in_maps


def gather_outputs(results: list):
    c_t = np.concatenate([results[i]["c_t"] for i in range(N_CORES)], axis=0)
    attn = np.concatenate([results[i]["attn"] for i in range(N_CORES)], axis=0)
    covn = np.concatenate(
        [results[i]["coverage_next"] for i in range(N_CORES)], axis=0
    )
    return c_t, attn, covn


def kernel(**inputs):
    nc = _get_nc()
    in_maps = make_in_maps(inputs)
    res = run_bass_kernel_spmd(nc, in_maps, core_ids=list(range(N_CORES)))
    return gather_outputs(res.results)


# revision 7
# speedup vs baseline: 1.0471x; 1.0471x over previous
"""Trainium2 Bass kernel for pointer-generator additive attention.

Full op (per batch b):
    dec_fea = s_t_hat @ W_d.T + b_d                         # (n,)
    att     = EF[b] + dec_fea[None,:] + cov[b][:,None]*W_c  # (t, n)
    score   = tanh(att) @ v                                 # (t,)
    attn    = renorm(softmax(score) * mask)                 # (t,)
    c_t     = attn @ EO[b]                                  # (n,)
    cov_next= cov + attn

Data-parallel over batch across 8 NeuronCores (8 batches/core, params
replicated, no collectives).

v2 design (vs the identity-matmul baseline):
  - EF/EO staged host-side as bf16 in the exact SBUF tile layout
    [119 partitions, 9 chunks x 1024] (t = c*119 + p, zero-padded): one
    fully-contiguous DMA per batch per tensor, half the HBM bytes.
  - T is tiled in chunks of 119 so K = 119 (identity band) + 8 (one-hot
    dec-row selectors) + 1 (cov -> W_c) = 128: a SINGLE matmul per
    (chunk, n-half) computes EF + dec_fea + cov*W_c fused.  The lhsT
    matrix [128, 8*1071] (identity + one-hots + cov row) is prebuilt on
    the host; dec_fea rows + W_c live at partitions 119..127 of the EF
    stream buffers (written once per buffer after the dec matvec).
  - ScalarE: one tanh per chunk (PSUM f32 -> SBUF bf16).
  - VectorE: scalar_tensor_tensor (th * v_bcast) with accum_out gives
    the n-reduction (score) per chunk.
  - Masked softmax + renorm + coverage batched in GROUPS OF 4 on
    [4, 1024] row tiles (one pass instead of per-batch row ops).
  - c_t: per chunk, lhsT = [119, 8] one-hot-column attn (column b =
    attn chunk, rest zero) so all 8 batches accumulate into a single
    [8, 512] x2 PSUM group; one ScalarE copy + one DMA at the end.
All heavy matmuls are bf16 (tolerance 2e-2, measured rel_err ~2.6e-3).
"""

import sys

if "/opt/trn_rl_repo" not in sys.path:
    sys.path.insert(0, "/opt/trn_rl_repo")

import ml_dtypes
import numpy as np

import concourse.bass as bass
import concourse.mybir as mybir
import concourse.tile as tile
from concourse import bacc
from concourse.bass_utils import run_bass_kernel_spmd
from concourse.masks import make_identity

F32 = mybir.dt.float32
BF16 = mybir.dt.bfloat16
AF = mybir.ActivationFunctionType
ALU = mybir.AluOpType

N_CORES = 8
B = 64
NB = B // N_CORES  # local batches per core
T = 1024
N = 1024
CH = 119           # t-chunk height (identity rows in the fused matmul)
NCH = 9            # chunks per batch: 8*119 + 72
LAST = T - (NCH - 1) * CH  # 72
W = NCH * CH       # 1071: per-batch window stride in lhsT
KT = N // 128      # k-tiles for the W_d matvec
GRP = 4            # softmax group size


def build_bass(nb: int = NB) -> bass.Bass:
    nc = bacc.Bacc()

    ef_d = nc.declare_dram_parameter("ef_tiles", [nb, CH * NCH, N], BF16, isOutput=False)
    eo_d = nc.declare_dram_parameter("eo_tiles", [nb, CH * NCH, N], BF16, isOutput=False)
    lhs_d = nc.declare_dram_parameter("lhsT_full", [128, nb * W], BF16, isOutput=False)
    mk_d = nc.declare_dram_parameter("enc_padding_mask", [nb, T], F32, isOutput=False)
    cv_d = nc.declare_dram_parameter("coverage", [nb, T], F32, isOutput=False)
    wdt_d = nc.declare_dram_parameter("W_d_T", [N, N], BF16, isOutput=False)
    st_d = nc.declare_dram_parameter("s_t_hat_T", [N, nb], BF16, isOutput=False)
    bd_d = nc.declare_dram_parameter("b_d", [N], BF16, isOutput=False)
    wc_d = nc.declare_dram_parameter("W_c", [N], BF16, isOutput=False)
    v_d = nc.declare_dram_parameter("v", [N], BF16, isOutput=False)
    ct_o = nc.declare_dram_parameter("c_t", [nb, N], F32, isOutput=True)
    at_o = nc.declare_dram_parameter("attn", [nb, T], F32, isOutput=True)
    cn_o = nc.declare_dram_parameter("coverage_next", [nb, T], F32, isOutput=True)

    with tile.TileContext(nc) as tc:
        with (
            tc.tile_pool(name="consts", bufs=1) as consts,
            tc.tile_pool(name="lhsp", bufs=1) as lhsp,
            tc.tile_pool(name="wdtp", bufs=1) as wdtp,
            tc.tile_pool(name="efp", bufs=2) as efp,
            tc.tile_pool(name="eop", bufs=3) as eop,
            tc.tile_pool(name="thp", bufs=3) as thp,
            tc.tile_pool(name="ttro", bufs=2) as ttro,
            tc.tile_pool(name="smal", bufs=2) as smal,
            tc.tile_pool(name="a9p", bufs=2) as a9p,
            tc.tile_pool(name="acwp", bufs=2) as acwp,
            tc.tile_pool(name="psA", bufs=2, space="PSUM") as psA,
            tc.tile_pool(name="psS", bufs=2, space="PSUM") as psS,
            tc.tile_pool(name="psT", bufs=2, space="PSUM") as psT,
        ):
            # ---------------- constants / small inputs ----------------
            ident = consts.tile([CH, CH], F32)
            make_identity(nc, ident)

            bd_b = consts.tile([1, N], BF16)
            nc.sync.dma_start(out=bd_b, in_=bd_d[None, :])
            v_b = consts.tile([1, N], BF16)
            nc.sync.dma_start(out=v_b, in_=v_d[None, :])
            wc_b = consts.tile([1, N], BF16)
            nc.sync.dma_start(out=wc_b, in_=wc_d[None, :])
            ones8 = consts.tile([1, NB], BF16)
            nc.vector.memset(ones8, 1.0)

            # full lhsT (identity band + one-hot dec selectors + cov row)
            lhs_all = lhsp.tile([128, nb * W], BF16)
            nc.gpsimd.dma_start(out=lhs_all, in_=lhs_d[:, :])

            # per-group f32 rows for softmax/coverage (engine APs must
            # start at a 32-aligned partition, so one [GRP, T] tile per group)
            ngrp = nb // GRP
            mask_g, cov_g, score_g, attn_g, covn_g = [], [], [], [], []
            for g in range(ngrp):
                mg = consts.tile([GRP, T], F32, name=f"mask{g}")
                nc.sync.dma_start(out=mg, in_=mk_d[g * GRP:(g + 1) * GRP, :])
                mask_g.append(mg)
                cg = consts.tile([GRP, T], F32, name=f"cov{g}")
                nc.sync.dma_start(out=cg, in_=cv_d[g * GRP:(g + 1) * GRP, :])
                cov_g.append(cg)
                score_g.append(consts.tile([GRP, T], F32, name=f"score{g}"))
                attn_g.append(consts.tile([GRP, T], F32, name=f"attn{g}"))
                covn_g.append(consts.tile([GRP, T], F32, name=f"covn{g}"))
            ct_sb = consts.tile([nb, N], F32)

            # v broadcast to all partitions for the score reduction
            v_bcast = consts.tile([128, N], BF16)
            nc.gpsimd.partition_broadcast(v_bcast, v_b)

            # W_d.T k-tiles (pre-transposed on the host), one DMA
            wdt_all = wdtp.tile([128, KT, N], BF16)
            nc.sync.dma_start(
                out=wdt_all, in_=wdt_d.rearrange("(kj p) n -> p kj n", p=128)
            )
            sT_all = consts.tile([128, KT, NB], BF16)
            nc.sync.dma_start(
                out=sT_all, in_=st_d.rearrange("(kj p) b -> p kj b", p=128)
            )

            # dec_fea rows = s_t_hat @ W_d.T + b_d   (bf16 matmuls, tiny)
            dec_rows = consts.tile([NB, N], BF16)
            for h in range(2):
                sl = slice(h * 512, (h + 1) * 512)
                psd = psA.tile([NB, 512], F32, tag="att")
                for kj in range(KT):
                    nc.tensor.matmul(
                        psd,
                        lhsT=sT_all[:, kj, :],
                        rhs=wdt_all[:, kj, sl],
                        start=(kj == 0), stop=False,
                    )
                nc.tensor.matmul(
                    psd, lhsT=ones8, rhs=bd_b[0:1, sl],
                    start=False, stop=True,
                )
                nc.scalar.activation(dec_rows[:, sl], psd, AF.Copy)

            # EF stream buffers: partitions 119..126 = dec rows,
            # partition 127 = W_c (constant across batches/chunks)
            ef_bufs = [efp.tile([128, NCH, N], BF16, tag="ef", name=f"efb{i}")
                       for i in range(2)]
            for buf in ef_bufs:
                for c in range(NCH):
                    nc.sync.dma_start(out=buf[119:127, c, :], in_=dec_rows)
                    nc.sync.dma_start(out=buf[127:128, c, :], in_=wc_b)

            # ---------------- pass 1: scores ----------------
            def chunk_m(c):
                return LAST if c == NCH - 1 else CH

            score_cols_t = {}

            def phase_a(b):
                buf = ef_bufs[b % 2]
                if b == 0:
                    # per-chunk loads so chunk 0 lands fast at kernel start
                    for c in range(NCH):
                        nc.gpsimd.dma_start(
                            out=buf[0:CH, c, :],
                            in_=ef_d[b, c * CH:(c + 1) * CH, :],
                        )
                else:
                    nc.gpsimd.dma_start(
                        out=buf[0:CH, :, :],
                        in_=ef_d[b, :, :].rearrange("(c p) n -> p c n", p=CH),
                    )

                score_cols = smal.tile([CH, NCH], F32, tag="scol")
                score_cols_t[b] = score_cols
                for c in range(NCH):
                    m = chunk_m(c)
                    att = psA.tile([CH, N], F32, tag="att")
                    for h in range(2):
                        nc.tensor.matmul(
                            att[0:m, h * 512:(h + 1) * 512],
                            lhsT=lhs_all[:, b * W + c * CH: b * W + c * CH + m],
                            rhs=buf[:, c, h * 512:(h + 1) * 512],
                            start=True, stop=True, skip_group_check=True,
                        )
                    th = thp.tile([CH, N], BF16, tag="th")
                    nc.scalar.activation(th[0:m, :], att[0:m, :], AF.Tanh)
                    scr = ttro.tile([CH, N], BF16, tag="ttro")
                    nc.vector.scalar_tensor_tensor(
                        out=scr[0:m, :], in0=th[0:m, :], scalar=1.0,
                        in1=v_bcast[0:m, :],
                        op0=ALU.mult, op1=ALU.mult,
                        accum_out=score_cols[0:m, c:c + 1],
                    )

                # score columns -> row b of score_all (t = c*119 + p)
                ps9 = psT.tile([NCH, CH], F32, tag="tscratch")
                nc.tensor.matmul(
                    ps9, lhsT=score_cols, rhs=ident, is_transpose=True,
                    start=True, stop=True,
                )
                score9 = smal.tile([NCH, CH], F32, tag="s9")
                nc.scalar.activation(score9, ps9, AF.Copy)
                srow = score_g[b // GRP]
                r = b % GRP
                nc.sync.dma_start(
                    out=srow[r:r + 1, 0:(NCH - 1) * CH].rearrange(
                        "p (c w) -> p c w", c=NCH - 1),
                    in_=score9[0:NCH - 1, :],
                )
                nc.sync.dma_start(
                    out=srow[r:r + 1, (NCH - 1) * CH:T],
                    in_=score9[NCH - 1:NCH, 0:LAST],
                )

            # masked softmax + renorm + coverage for GRP batches at once
            def softmax_group(g):
                # scores are O(1) (|s| < ~3): plain exp is safe, skip max-sub
                nc.scalar.activation(attn_g[g], score_g[g], AF.Exp)
                ssum = smal.tile([GRP, 1], F32, tag="ssum")
                nc.vector.scalar_tensor_tensor(
                    out=attn_g[g], in0=attn_g[g], scalar=1.0,
                    in1=mask_g[g],
                    op0=ALU.mult, op1=ALU.mult, accum_out=ssum,
                )
                rs = smal.tile([GRP, 1], F32, tag="rs")
                nc.vector.reciprocal(rs, ssum)
                nc.vector.tensor_scalar_mul(attn_g[g], attn_g[g], rs)
                nc.vector.tensor_add(covn_g[g], cov_g[g], attn_g[g])

            # ---------------- pass 2: context vectors ----------------
            eo_bufs = {}

            def load_eo(b):
                buf = eop.tile([CH, NCH, N], BF16, tag="eo")
                nc.gpsimd.dma_start(
                    out=buf, in_=eo_d[b, :, :].rearrange("(c p) n -> p c n", p=CH)
                )
                eo_bufs[b] = buf

            ctps = [psS.tile([NB, 512], F32, tag="srow", name=f"ctp{h}")
                    for h in range(2)]

            def phase_c(b):
                # attn row -> [9, 119] -> transpose -> one-hot column b
                attn9 = a9p.tile([NCH, CH], F32, tag="attn9")
                arow = attn_g[b // GRP]
                r = b % GRP
                nc.sync.dma_start(
                    out=attn9[0:NCH - 1, :],
                    in_=arow[r:r + 1, 0:(NCH - 1) * CH].rearrange(
                        "p (c w) -> p c w", c=NCH - 1),
                )
                nc.sync.dma_start(
                    out=attn9[NCH - 1:NCH, 0:LAST],
                    in_=arow[r:r + 1, (NCH - 1) * CH:T],
                )
                acp = psT.tile([CH, NCH], F32, tag="tscratch")
                nc.tensor.matmul(
                    acp, lhsT=attn9, rhs=ident[0:NCH, 0:NCH],
                    is_transpose=True, start=True, stop=True,
                )
                acw = acwp.tile([CH, NCH, NB], BF16, tag="acw")
                nc.gpsimd.memset(acw, 0.0)
                nc.scalar.activation(acw[:, :, b], acp, AF.Copy)

                buf = eo_bufs.pop(b)
                for c in range(NCH):
                    m = chunk_m(c)
                    for h in range(2):
                        nc.tensor.matmul(
                            ctps[h],
                            lhsT=acw[0:m, c, :],
                            rhs=buf[0:m, c, h * 512:(h + 1) * 512],
                            start=(b == 0 and c == 0),
                            stop=(b == nb - 1 and c == NCH - 1),
                            skip_group_check=True,
                        )

            # ---------------- schedule ----------------
            # pass-1 front, softmax per half, pass-2 interleaved one group back
            for b in range(GRP):
                phase_a(b)
                if b >= 2:
                    load_eo(b - 2)
            softmax_group(0)
            for b in range(GRP, nb):
                phase_a(b)
                load_eo(b - 2)
                phase_c(b - GRP)
            softmax_group(1)
            load_eo(nb - 2)
            load_eo(nb - 1)
            for b in range(GRP, nb):
                phase_c(b)

            for h in range(2):
                nc.scalar.activation(
                    ct_sb[:, h * 512:(h + 1) * 512], ctps[h], AF.Copy
                )
            nc.sync.dma_start(out=ct_o[:, :], in_=ct_sb)
            for g in range(ngrp):
                rows = slice(g * GRP, (g + 1) * GRP)
                nc.sync.dma_start(out=at_o[rows, :], in_=attn_g[g])
                nc.sync.dma_start(out=cn_o[rows, :], in_=covn_g[g])

    nc.finalize()
    return nc


_CACHE: dict = {}


def _get_nc() -> bass.Bass:
    if "nc" not in _CACHE:
        _CACHE["nc"] = build_bass(NB)
    return _CACHE["nc"]


def _stage_tiles(x):
    """[T, N] f32 -> [1071, N] bf16 natural rows, zero-padded."""
    pad = np.zeros((CH * NCH, N), np.float32)
    pad[:T] = x
    return pad.astype(ml_dtypes.bfloat16)


def _build_lhs(cov_core):
    """[128, 8*1071] bf16: identity band + one-hot rows + cov row."""
    lhs = np.zeros((128, NB * W), np.float32)
    eye = np.eye(CH, dtype=np.float32)
    lhs[:CH] = np.tile(eye, (1, NB * NCH))
    for j in range(NB):
        lhs[CH + j, j * W:(j + 1) * W] = 1.0
        lhs[127, j * W:j * W + T] = cov_core[j]
    return lhs.astype(ml_dtypes.bfloat16)


def make_in_maps(inputs: dict) -> list:
    f = lambda x: np.ascontiguousarray(np.asarray(x), dtype=np.float32)
    s = f(inputs["s_t_hat"])
    eo = f(inputs["encoder_outputs"])
    ef = f(inputs["encoder_feature"]).reshape(B, T, N)
    mk = f(inputs["enc_padding_mask"])
    cv = f(inputs["coverage"])
    wdt = np.ascontiguousarray(f(inputs["W_d"]).T).astype(ml_dtypes.bfloat16)
    bd = f(inputs["b_d"])
    wc = f(inputs["W_c"])
    vv = f(inputs["v"])
    in_maps = []
    for i in range(N_CORES):
        sl = slice(i * NB, (i + 1) * NB)
        ef_tiles = np.stack([_stage_tiles(ef[i * NB + j]) for j in range(NB)])
        eo_tiles = np.stack([_stage_tiles(eo[i * NB + j]) for j in range(NB)])
        in_maps.append({
            "ef_tiles": ef_tiles,
            "eo_tiles": eo_tiles,
            "lhsT_full": _build_lhs(cv[sl]),
            "s_t_hat_T": np.ascontiguousarray(s[sl].T).astype(ml_dtypes.bfloat16),
            "enc_padding_mask": mk[sl],
            "coverage": cv[sl],
            "W_d_T": wdt,
            "b_d": bd.astype(ml_dtypes.bfloat16),
            "W_c": wc.astype(ml_dtypes.bfloat16),
            "v": vv.astype(ml_dtypes.bfloat16),
        })
    return in_maps


def gather_outputs(results: list):
    c_t = np.concatenate([results[i]["c_t"] for i in range(N_CORES)], axis=0)
    attn = np.concatenate([results[i]["attn"] for i in range(N_CORES)], axis=0)
    covn = np.concatenate(
        [results[i]["coverage_next"] for i in range(N_CORES)], axis=0
    )
    return c_t, attn, covn


def kernel(**inputs):
    nc = _get_nc()
    in_maps = make_in_maps(inputs)
    res = run_bass_kernel_spmd(nc, in_maps, core_ids=list(range(N_CORES)))
    return gather_outputs(res.results)


# revision 9
# speedup vs baseline: 1.0554x; 1.0080x over previous
"""Trainium2 Bass kernel for pointer-generator additive attention.

Full op (per batch b):
    dec_fea = s_t_hat @ W_d.T + b_d                         # (n,)
    att     = EF[b] + dec_fea[None,:] + cov[b][:,None]*W_c  # (t, n)
    score   = tanh(att) @ v                                 # (t,)
    attn    = renorm(softmax(score) * mask)                 # (t,)
    c_t     = attn @ EO[b]                                  # (n,)
    cov_next= cov + attn

Data-parallel over batch across 8 NeuronCores (8 batches/core, params
replicated, no collectives).

v2 design (vs the identity-matmul baseline):
  - EF/EO staged host-side as bf16 in the exact SBUF tile layout
    [119 partitions, 9 chunks x 1024] (t = c*119 + p, zero-padded): one
    fully-contiguous DMA per batch per tensor, half the HBM bytes.
  - T is tiled in chunks of 119 so K = 119 (identity band) + 8 (one-hot
    dec-row selectors) + 1 (cov -> W_c) = 128: a SINGLE matmul per
    (chunk, n-half) computes EF + dec_fea + cov*W_c fused.  The lhsT
    matrix [128, 8*1071] (identity + one-hots + cov row) is prebuilt on
    the host; dec_fea rows + W_c live at partitions 119..127 of the EF
    stream buffers (written once per buffer after the dec matvec).
  - ScalarE: one tanh per chunk (PSUM f32 -> SBUF bf16).
  - VectorE: scalar_tensor_tensor (th * v_bcast) with accum_out gives
    the n-reduction (score) per chunk.
  - Masked softmax + renorm + coverage batched in GROUPS OF 4 on
    [4, 1024] row tiles (one pass instead of per-batch row ops).
  - c_t: per chunk, lhsT = [119, 8] one-hot-column attn (column b =
    attn chunk, rest zero) so all 8 batches accumulate into a single
    [8, 512] x2 PSUM group; one ScalarE copy + one DMA at the end.
All heavy matmuls are bf16 (tolerance 2e-2, measured rel_err ~2.6e-3).
"""

import sys

if "/opt/trn_rl_repo" not in sys.path:
    sys.path.insert(0, "/opt/trn_rl_repo")

import ml_dtypes
import numpy as np

import concourse.bass as bass
import concourse.mybir as mybir
import concourse.tile as tile
from concourse import bacc
from concourse.bass_utils import run_bass_kernel_spmd
from concourse.masks import make_identity

F32 = mybir.dt.float32
BF16 = mybir.dt.bfloat16
AF = mybir.ActivationFunctionType
ALU = mybir.AluOpType

N_CORES = 8
B = 64
NB = B // N_CORES  # local batches per core
T = 1024
N = 1024
CH = 119           # t-chunk height (identity rows in the fused matmul)
NCH = 9            # chunks per batch: 8*119 + 72
LAST = T - (NCH - 1) * CH  # 72
W = NCH * CH       # 1071: per-batch window stride in lhsT
KT = N // 128      # k-tiles for the W_d matvec
GRP = 8            # softmax group size (single group)


def build_bass(nb: int = NB) -> bass.Bass:
    nc = bacc.Bacc()

    ef_d = nc.declare_dram_parameter("ef_tiles", [nb, CH, NCH * N], BF16, isOutput=False)
    eo_d = nc.declare_dram_parameter("eo_tiles", [nb, CH, NCH * N], BF16, isOutput=False)
    lhsA_d = nc.declare_dram_parameter("lhsT_a", [4, 128, 2048], BF16, isOutput=False)
    lhsB_d = nc.declare_dram_parameter("lhsT_b", [128, nb * W - 8192], BF16, isOutput=False)
    mk_d = nc.declare_dram_parameter("enc_padding_mask", [nb, T], F32, isOutput=False)
    cv_d = nc.declare_dram_parameter("coverage", [nb, T], F32, isOutput=False)
    wdt_d = nc.declare_dram_parameter("W_d_T", [4, 128, 2048], BF16, isOutput=False)
    st_d = nc.declare_dram_parameter("s_t_hat_T", [N, nb], BF16, isOutput=False)
    bd_d = nc.declare_dram_parameter("b_d", [N], BF16, isOutput=False)
    wc_d = nc.declare_dram_parameter("W_c", [N], BF16, isOutput=False)
    v_d = nc.declare_dram_parameter("v", [N], BF16, isOutput=False)
    ct_o = nc.declare_dram_parameter("c_t", [nb, N], F32, isOutput=True)
    at_o = nc.declare_dram_parameter("attn", [nb, T], F32, isOutput=True)
    cn_o = nc.declare_dram_parameter("coverage_next", [nb, T], F32, isOutput=True)

    with tile.TileContext(nc) as tc:
        with (
            tc.tile_pool(name="consts", bufs=1) as consts,
            tc.tile_pool(name="lhsp", bufs=1) as lhsp,
            tc.tile_pool(name="wdtp", bufs=1) as wdtp,
            tc.tile_pool(name="efp", bufs=2) as efp,
            tc.tile_pool(name="eop", bufs=3) as eop,
            tc.tile_pool(name="thp", bufs=3) as thp,
            tc.tile_pool(name="ttro", bufs=2) as ttro,
            tc.tile_pool(name="smal", bufs=2) as smal,
            tc.tile_pool(name="a9p", bufs=2) as a9p,
            tc.tile_pool(name="acwp", bufs=2) as acwp,
            tc.tile_pool(name="psA", bufs=2, space="PSUM") as psA,
            tc.tile_pool(name="psS", bufs=2, space="PSUM") as psS,
            tc.tile_pool(name="psT", bufs=2, space="PSUM") as psT,
        ):
            # ---------------- constants / small inputs ----------------
            ident = consts.tile([CH, CH], F32)
            make_identity(nc, ident)

            bd_b = consts.tile([1, N], BF16)
            nc.sync.dma_start(out=bd_b, in_=bd_d[None, :])
            v_b = consts.tile([1, N], BF16)
            nc.sync.dma_start(out=v_b, in_=v_d[None, :])
            wc_b = consts.tile([1, N], BF16)
            nc.sync.dma_start(out=wc_b, in_=wc_d[None, :])
            ones8 = consts.tile([1, NB], BF16)
            nc.vector.memset(ones8, 1.0)

            # full lhsT (identity band + one-hot dec selectors + cov row)
            lhs_all = lhsp.tile([128, nb * W], BF16)
            nc.gpsimd.dma_start(
                out=lhs_all[:, 0:8192].rearrange("p (c m) -> p c m", c=4),
                in_=lhsA_d[:, :, :].rearrange("c p m -> p c m"),
            )
            nc.gpsimd.dma_start(out=lhs_all[:, 8192:], in_=lhsB_d[:, :])

            # per-group f32 rows for softmax/coverage (engine APs must
            # start at a 32-aligned partition, so one [GRP, T] tile per group)
            ngrp = nb // GRP
            mask_g, cov_g, score_g, attn_g, covn_g = [], [], [], [], []
            for g in range(ngrp):
                mg = consts.tile([GRP, T], F32, name=f"mask{g}")
                nc.sync.dma_start(out=mg, in_=mk_d[g * GRP:(g + 1) * GRP, :])
                mask_g.append(mg)
                cg = consts.tile([GRP, T], F32, name=f"cov{g}")
                nc.sync.dma_start(out=cg, in_=cv_d[g * GRP:(g + 1) * GRP, :])
                cov_g.append(cg)
                score_g.append(consts.tile([GRP, T], F32, name=f"score{g}"))
                attn_g.append(consts.tile([GRP, T], F32, name=f"attn{g}"))
                covn_g.append(consts.tile([GRP, T], F32, name=f"covn{g}"))
            ct_sb = consts.tile([nb, N], F32)

            # v broadcast to all partitions for the score reduction
            v_bcast = consts.tile([128, N], BF16)
            nc.gpsimd.partition_broadcast(v_bcast, v_b)

            # W_d.T k-tiles (pre-transposed, pair-packed on the host)
            wdt_all = wdtp.tile([128, KT, N], BF16)
            nc.sync.dma_start(
                out=wdt_all.rearrange("p (c k) n -> p c (k n)", c=4),
                in_=wdt_d[:, :, :].rearrange("c p m -> p c m"),
            )
            sT_all = consts.tile([128, KT, NB], BF16)
            nc.sync.dma_start(
                out=sT_all, in_=st_d.rearrange("(kj p) b -> p kj b", p=128)
            )

            # dec_fea rows = s_t_hat @ W_d.T + b_d   (bf16 matmuls, tiny)
            dec_rows = consts.tile([NB, N], BF16)
            for h in range(2):
                sl = slice(h * 512, (h + 1) * 512)
                psd = psA.tile([NB, 512], F32, tag="att")
                for kj in range(KT):
                    nc.tensor.matmul(
                        psd,
                        lhsT=sT_all[:, kj, :],
                        rhs=wdt_all[:, kj, sl],
                        start=(kj == 0), stop=False,
                    )
                nc.tensor.matmul(
                    psd, lhsT=ones8, rhs=bd_b[0:1, sl],
                    start=False, stop=True,
                )
                nc.scalar.activation(dec_rows[:, sl], psd, AF.Copy)

            # EF stream buffers: partitions 119..126 = dec rows,
            # partition 127 = W_c (constant across batches/chunks)
            ef_bufs = [efp.tile([128, NCH, N], BF16, tag="ef", name=f"efb{i}")
                       for i in range(2)]
            for buf in ef_bufs:
                for c in range(NCH):
                    nc.sync.dma_start(out=buf[119:127, c, :], in_=dec_rows)
                    nc.sync.dma_start(out=buf[127:128, c, :], in_=wc_b)

            # ---------------- pass 1: scores ----------------
            def chunk_m(c):
                return LAST if c == NCH - 1 else CH

            score_cols_t = {}

            def phase_a(b):
                buf = ef_bufs[b % 2]
                if b == 0:
                    # per-pair loads so the first chunks land fast at start
                    for c2 in range(4):
                        nc.gpsimd.dma_start(
                            out=buf[0:CH, 2 * c2:2 * c2 + 2, :],
                            in_=ef_d[b, :, c2 * 2048:(c2 + 1) * 2048],
                        )
                    nc.gpsimd.dma_start(
                        out=buf[0:CH, NCH - 1, :], in_=ef_d[b, :, 8192:9216]
                    )
                else:
                    nc.gpsimd.dma_start(
                        out=buf[0:CH, 0:NCH - 1, :],
                        in_=ef_d[b, :, 0:8192].rearrange("p (c m) -> p c m", c=4),
                    )
                    nc.gpsimd.dma_start(
                        out=buf[0:CH, NCH - 1, :], in_=ef_d[b, :, 8192:9216]
                    )

                score_cols = smal.tile([CH, NCH], F32, tag="scol")
                score_cols_t[b] = score_cols
                for c in range(NCH):
                    m = chunk_m(c)
                    att = psA.tile([CH, N], F32, tag="att")
                    for h in range(2):
                        nc.tensor.matmul(
                            att[0:m, h * 512:(h + 1) * 512],
                            lhsT=lhs_all[:, b * W + c * CH: b * W + c * CH + m],
                            rhs=buf[:, c, h * 512:(h + 1) * 512],
                            start=True, stop=True, skip_group_check=True,
                        )
                    th = thp.tile([CH, N], BF16, tag="th")
                    nc.scalar.activation(th[0:m, :], att[0:m, :], AF.Tanh)
                    scr = ttro.tile([CH, N], BF16, tag="ttro")
                    nc.vector.scalar_tensor_tensor(
                        out=scr[0:m, :], in0=th[0:m, :], scalar=1.0,
                        in1=v_bcast[0:m, :],
                        op0=ALU.mult, op1=ALU.mult,
                        accum_out=score_cols[0:m, c:c + 1],
                    )

                # score columns -> row b of score_all (t = c*119 + p)
                ps9 = psT.tile([NCH, CH], F32, tag="tscratch")
                nc.tensor.matmul(
                    ps9, lhsT=score_cols, rhs=ident, is_transpose=True,
                    start=True, stop=True,
                )
                score9 = smal.tile([NCH, CH], F32, tag="s9")
                nc.scalar.activation(score9, ps9, AF.Copy)
                srow = score_g[b // GRP]
                r = b % GRP
                nc.sync.dma_start(
                    out=srow[r:r + 1, 0:(NCH - 1) * CH].rearrange(
                        "p (c w) -> p c w", c=NCH - 1),
                    in_=score9[0:NCH - 1, :],
                )
                nc.sync.dma_start(
                    out=srow[r:r + 1, (NCH - 1) * CH:T],
                    in_=score9[NCH - 1:NCH, 0:LAST],
                )

            # masked softmax + renorm + coverage for GRP batches at once
            def softmax_group(g):
                # scores are O(1) (|s| < ~3): plain exp is safe, skip max-sub
                nc.scalar.activation(attn_g[g], score_g[g], AF.Exp)
                ssum = smal.tile([GRP, 1], F32, tag="ssum")
                nc.vector.scalar_tensor_tensor(
                    out=attn_g[g], in0=attn_g[g], scalar=1.0,
                    in1=mask_g[g],
                    op0=ALU.mult, op1=ALU.mult, accum_out=ssum,
                )
                rs = smal.tile([GRP, 1], F32, tag="rs")
                nc.vector.reciprocal(rs, ssum)
                nc.vector.tensor_scalar_mul(attn_g[g], attn_g[g], rs)
                nc.vector.tensor_add(covn_g[g], cov_g[g], attn_g[g])

            # ---------------- pass 2: context vectors ----------------
            eo_bufs = {}

            def load_eo(b):
                buf = eop.tile([CH, NCH, N], BF16, tag="eo")
                nc.gpsimd.dma_start(
                    out=buf[:, 0:NCH - 1, :],
                    in_=eo_d[b, :, 0:8192].rearrange("p (c m) -> p c m", c=4),
                )
                nc.gpsimd.dma_start(
                    out=buf[:, NCH - 1, :], in_=eo_d[b, :, 8192:9216]
                )
                eo_bufs[b] = buf

            ctps = [psS.tile([NB, 512], F32, tag="srow", name=f"ctp{h}")
                    for h in range(2)]

            def phase_c(b):
                # attn row -> [9, 119] -> transpose -> one-hot column b
                attn9 = a9p.tile([NCH, CH], F32, tag="attn9")
                arow = attn_g[b // GRP]
                r = b % GRP
                nc.sync.dma_start(
                    out=attn9[0:NCH - 1, :],
                    in_=arow[r:r + 1, 0:(NCH - 1) * CH].rearrange(
                        "p (c w) -> p c w", c=NCH - 1),
                )
                nc.sync.dma_start(
                    out=attn9[NCH - 1:NCH, 0:LAST],
                    in_=arow[r:r + 1, (NCH - 1) * CH:T],
                )
                acp = psT.tile([CH, NCH], F32, tag="tscratch")
                nc.tensor.matmul(
                    acp, lhsT=attn9, rhs=ident[0:NCH, 0:NCH],
                    is_transpose=True, start=True, stop=True,
                )
                acw = acwp.tile([CH, NCH, NB], BF16, tag="acw")
                nc.gpsimd.memset(acw, 0.0)
                nc.scalar.activation(acw[:, :, b], acp, AF.Copy)

                buf = eo_bufs.pop(b)
                for c in range(NCH):
                    m = chunk_m(c)
                    for h in range(2):
                        nc.tensor.matmul(
                            ctps[h],
                            lhsT=acw[0:m, c, :],
                            rhs=buf[0:m, c, h * 512:(h + 1) * 512],
                            start=(b == 0 and c == 0),
                            stop=(b == nb - 1 and c == NCH - 1),
                            skip_group_check=True,
                        )

            # ---------------- schedule ----------------
            # pass 1 (scores), one softmax, pass 2 (context vectors);
            # EO prefetch limited to pool depth to avoid queue blocking
            for b in range(nb):
                phase_a(b)
                if 2 <= b <= 4:
                    load_eo(b - 2)
            softmax_group(0)
            for b in range(nb):
                phase_c(b)
                if b + 3 < nb:
                    load_eo(b + 3)

            for h in range(2):
                nc.scalar.activation(
                    ct_sb[:, h * 512:(h + 1) * 512], ctps[h], AF.Copy
                )
            nc.sync.dma_start(out=ct_o[:, :], in_=ct_sb)
            for g in range(ngrp):
                rows = slice(g * GRP, (g + 1) * GRP)
                nc.sync.dma_start(out=at_o[rows, :], in_=attn_g[g])
                nc.sync.dma_start(out=cn_o[rows, :], in_=covn_g[g])

    nc.finalize()
    return nc


_CACHE: dict = {}


def _get_nc() -> bass.Bass:
    if "nc" not in _CACHE:
        _CACHE["nc"] = build_bass(NB)
    return _CACHE["nc"]


def _stage_tiles(x):
    """[T, N] f32 -> [119, 9216] bf16: chunk-pair layout (4KB DMA lines).

    Row p holds chunks c=0..8 of the SBUF tile (t = c*119 + p), with
    chunk pairs (2c2, 2c2+1) adjacent so DMA descriptors move 4KB."""
    pad = np.zeros((CH * NCH, N), np.float32)
    pad[:T] = x
    a = pad.reshape(NCH, CH, N)
    main = a[0:NCH - 1].transpose(1, 0, 2).reshape(CH, (NCH - 1) * N)
    tail = a[NCH - 1]
    return np.concatenate([main, tail], axis=1).astype(ml_dtypes.bfloat16)


def _build_lhs(cov_core):
    """identity band + one-hot rows + cov row, split into 4KB-line part A
    ([4, 128, 2048], cols 0..8191 pair-packed) and tail part B."""
    lhs = np.zeros((128, NB * W), np.float32)
    eye = np.eye(CH, dtype=np.float32)
    lhs[:CH] = np.tile(eye, (1, NB * NCH))
    for j in range(NB):
        lhs[CH + j, j * W:(j + 1) * W] = 1.0
        lhs[127, j * W:j * W + T] = cov_core[j]
    lhs = lhs.astype(ml_dtypes.bfloat16)
    a = np.ascontiguousarray(lhs[:, 0:8192].reshape(128, 4, 2048).transpose(1, 0, 2))
    b = np.ascontiguousarray(lhs[:, 8192:])
    return a, b


def make_in_maps(inputs: dict) -> list:
    f = lambda x: np.ascontiguousarray(np.asarray(x), dtype=np.float32)
    s = f(inputs["s_t_hat"])
    eo = f(inputs["encoder_outputs"])
    ef = f(inputs["encoder_feature"]).reshape(B, T, N)
    mk = f(inputs["enc_padding_mask"])
    cv = f(inputs["coverage"])
    wdt = np.ascontiguousarray(f(inputs["W_d"]).T).astype(ml_dtypes.bfloat16)
    # [1024, 1024] -> k-tile pair layout [4, 128, 2048] (4KB DMA lines):
    # wdt_all[p, kj, :] = W_d_T[kj*128 + p, :]
    wdt_pairs = np.ascontiguousarray(
        wdt.reshape(KT, 128, N).transpose(1, 0, 2).reshape(128, 4, 2048)
        .transpose(1, 0, 2)
    )
    bd = f(inputs["b_d"])
    wc = f(inputs["W_c"])
    vv = f(inputs["v"])
    in_maps = []
    for i in range(N_CORES):
        sl = slice(i * NB, (i + 1) * NB)
        ef_tiles = np.stack([_stage_tiles(ef[i * NB + j]) for j in range(NB)])
        eo_tiles = np.stack([_stage_tiles(eo[i * NB + j]) for j in range(NB)])
        lhs_a, lhs_b = _build_lhs(cv[sl])
        in_maps.append({
            "ef_tiles": ef_tiles,
            "eo_tiles": eo_tiles,
            "lhsT_a": lhs_a,
            "lhsT_b": lhs_b,
            "s_t_hat_T": np.ascontiguousarray(s[sl].T).astype(ml_dtypes.bfloat16),
            "enc_padding_mask": mk[sl],
            "coverage": cv[sl],
            "W_d_T": wdt_pairs,
            "b_d": bd.astype(ml_dtypes.bfloat16),
            "W_c": wc.astype(ml_dtypes.bfloat16),
            "v": vv.astype(ml_dtypes.bfloat16),
        })
    return in_maps


def gather_outputs(results: list):
    c_t = np.concatenate([results[i]["c_t"] for i in range(N_CORES)], axis=0)
    attn = np.concatenate([results[i]["attn"] for i in range(N_CORES)], axis=0)
    covn = np.concatenate(
        [results[i]["coverage_next"] for i in range(N_CORES)], axis=0
    )
    return c_t, attn, covn


def kernel(**inputs):
    nc = _get_nc()
    in_maps = make_in_maps(inputs)
    res = run_bass_kernel_spmd(nc, in_maps, core_ids=list(range(N_CORES)))
    return gather_outputs(res.results)


# revision 10
# speedup vs baseline: 1.0573x; 1.0018x over previous
"""Trainium2 Bass kernel for pointer-generator additive attention.

Full op (per batch b):
    dec_fea = s_t_hat @ W_d.T + b_d                         # (n,)
    att     = EF[b] + dec_fea[None,:] + cov[b][:,None]*W_c  # (t, n)
    score   = tanh(att) @ v                                 # (t,)
    attn    = renorm(softmax(score) * mask)                 # (t,)
    c_t     = attn @ EO[b]                                  # (n,)
    cov_next= cov + attn

Data-parallel over batch across 8 NeuronCores (8 batches/core, params
replicated, no collectives).

v2 design (vs the identity-matmul baseline):
  - EF/EO staged host-side as bf16 in the exact SBUF tile layout
    [119 partitions, 9 chunks x 1024] (t = c*119 + p, zero-padded): one
    fully-contiguous DMA per batch per tensor, half the HBM bytes.
  - T is tiled in chunks of 119 so K = 119 (identity band) + 8 (one-hot
    dec-row selectors) + 1 (cov -> W_c) = 128: a SINGLE matmul per
    (chunk, n-half) computes EF + dec_fea + cov*W_c fused.  The lhsT
    matrix [128, 8*1071] (identity + one-hots + cov row) is prebuilt on
    the host; dec_fea rows + W_c live at partitions 119..127 of the EF
    stream buffers (written once per buffer after the dec matvec).
  - ScalarE: one tanh per chunk (PSUM f32 -> SBUF bf16).
  - VectorE: scalar_tensor_tensor (th * v_bcast) with accum_out gives
    the n-reduction (score) per chunk.
  - Masked softmax + renorm + coverage batched in GROUPS OF 4 on
    [4, 1024] row tiles (one pass instead of per-batch row ops).
  - c_t: per chunk, lhsT = [119, 8] one-hot-column attn (column b =
    attn chunk, rest zero) so all 8 batches accumulate into a single
    [8, 512] x2 PSUM group; one ScalarE copy + one DMA at the end.
All heavy matmuls are bf16 (tolerance 2e-2, measured rel_err ~2.6e-3).
"""

import sys

if "/opt/trn_rl_repo" not in sys.path:
    sys.path.insert(0, "/opt/trn_rl_repo")

import ml_dtypes
import numpy as np

import concourse.bass as bass
import concourse.mybir as mybir
import concourse.tile as tile
from concourse import bacc
from concourse.bass_utils import run_bass_kernel_spmd
from concourse.masks import make_identity

F32 = mybir.dt.float32
BF16 = mybir.dt.bfloat16
AF = mybir.ActivationFunctionType
ALU = mybir.AluOpType

N_CORES = 8
B = 64
NB = B // N_CORES  # local batches per core
T = 1024
N = 1024
CH = 119           # t-chunk height (identity rows in the fused matmul)
NCH = 9            # chunks per batch: 8*119 + 72
LAST = T - (NCH - 1) * CH  # 72
W = NCH * CH       # 1071: per-batch window stride in lhsT
KT = N // 128      # k-tiles for the W_d matvec
GRP = 8            # softmax group size (single group)


def build_bass(nb: int = NB) -> bass.Bass:
    nc = bacc.Bacc()

    efA_d = nc.declare_dram_parameter("ef_a", [nb, 4, CH, 2 * N], BF16, isOutput=False)
    efB_d = nc.declare_dram_parameter("ef_b", [nb, CH, N], BF16, isOutput=False)
    eoA_d = nc.declare_dram_parameter("eo_a", [nb, 4, CH, 2 * N], BF16, isOutput=False)
    eoB_d = nc.declare_dram_parameter("eo_b", [nb, CH, N], BF16, isOutput=False)
    lhsA_d = nc.declare_dram_parameter("lhsT_a", [4, 128, 2048], BF16, isOutput=False)
    lhsB_d = nc.declare_dram_parameter("lhsT_b", [128, nb * W - 8192], BF16, isOutput=False)
    mk_d = nc.declare_dram_parameter("enc_padding_mask", [nb, T], F32, isOutput=False)
    cv_d = nc.declare_dram_parameter("coverage", [nb, T], F32, isOutput=False)
    wdt_d = nc.declare_dram_parameter("W_d_T", [4, 128, 2048], BF16, isOutput=False)
    st_d = nc.declare_dram_parameter("s_t_hat_T", [N, nb], BF16, isOutput=False)
    bd_d = nc.declare_dram_parameter("b_d", [N], BF16, isOutput=False)
    wc_d = nc.declare_dram_parameter("W_c", [N], BF16, isOutput=False)
    v_d = nc.declare_dram_parameter("v", [N], BF16, isOutput=False)
    ct_o = nc.declare_dram_parameter("c_t", [nb, N], F32, isOutput=True)
    at_o = nc.declare_dram_parameter("attn", [nb, T], F32, isOutput=True)
    cn_o = nc.declare_dram_parameter("coverage_next", [nb, T], F32, isOutput=True)

    with tile.TileContext(nc) as tc:
        with (
            tc.tile_pool(name="consts", bufs=1) as consts,
            tc.tile_pool(name="lhsp", bufs=1) as lhsp,
            tc.tile_pool(name="wdtp", bufs=1) as wdtp,
            tc.tile_pool(name="efp", bufs=2) as efp,
            tc.tile_pool(name="eop", bufs=3) as eop,
            tc.tile_pool(name="thp", bufs=3) as thp,
            tc.tile_pool(name="ttro", bufs=2) as ttro,
            tc.tile_pool(name="smal", bufs=2) as smal,
            tc.tile_pool(name="a9p", bufs=2) as a9p,
            tc.tile_pool(name="acwp", bufs=2) as acwp,
            tc.tile_pool(name="psA", bufs=2, space="PSUM") as psA,
            tc.tile_pool(name="psS", bufs=2, space="PSUM") as psS,
            tc.tile_pool(name="psT", bufs=2, space="PSUM") as psT,
        ):
            # ---------------- constants / small inputs ----------------
            ident = consts.tile([CH, CH], F32)
            make_identity(nc, ident)

            bd_b = consts.tile([1, N], BF16)
            nc.sync.dma_start(out=bd_b, in_=bd_d[None, :])
            v_b = consts.tile([1, N], BF16)
            nc.sync.dma_start(out=v_b, in_=v_d[None, :])
            wc_b = consts.tile([1, N], BF16)
            nc.sync.dma_start(out=wc_b, in_=wc_d[None, :])
            ones8 = consts.tile([1, NB], BF16)
            nc.vector.memset(ones8, 1.0)

            # full lhsT (identity band + one-hot dec selectors + cov row)
            lhs_all = lhsp.tile([128, nb * W], BF16)
            nc.gpsimd.dma_start(
                out=lhs_all[:, 0:8192].rearrange("p (c m) -> p c m", c=4),
                in_=lhsA_d[:, :, :].rearrange("c p m -> p c m"),
            )
            nc.gpsimd.dma_start(out=lhs_all[:, 8192:], in_=lhsB_d[:, :])

            # per-group f32 rows for softmax/coverage (engine APs must
            # start at a 32-aligned partition, so one [GRP, T] tile per group)
            ngrp = nb // GRP
            mask_g, cov_g, score_g, attn_g, covn_g = [], [], [], [], []
            for g in range(ngrp):
                mg = consts.tile([GRP, T], F32, name=f"mask{g}")
                nc.sync.dma_start(out=mg, in_=mk_d[g * GRP:(g + 1) * GRP, :])
                mask_g.append(mg)
                cg = consts.tile([GRP, T], F32, name=f"cov{g}")
                nc.sync.dma_start(out=cg, in_=cv_d[g * GRP:(g + 1) * GRP, :])
                cov_g.append(cg)
                score_g.append(consts.tile([GRP, T], F32, name=f"score{g}"))
                attn_g.append(consts.tile([GRP, T], F32, name=f"attn{g}"))
                covn_g.append(consts.tile([GRP, T], F32, name=f"covn{g}"))
            ct_sb = consts.tile([nb, N], F32)

            # v broadcast to all partitions for the score reduction
            v_bcast = consts.tile([128, N], BF16)
            nc.gpsimd.partition_broadcast(v_bcast, v_b)

            # W_d.T k-tiles (pre-transposed, pair-packed on the host)
            wdt_all = wdtp.tile([128, KT, N], BF16)
            nc.sync.dma_start(
                out=wdt_all.rearrange("p (c k) n -> p c (k n)", c=4),
                in_=wdt_d[:, :, :].rearrange("c p m -> p c m"),
            )
            sT_all = consts.tile([128, KT, NB], BF16)
            nc.sync.dma_start(
                out=sT_all, in_=st_d.rearrange("(kj p) b -> p kj b", p=128)
            )

            # dec_fea rows = s_t_hat @ W_d.T + b_d   (bf16 matmuls, tiny)
            dec_rows = consts.tile([NB, N], BF16)
            for h in range(2):
                sl = slice(h * 512, (h + 1) * 512)
                psd = psA.tile([NB, 512], F32, tag="att")
                for kj in range(KT):
                    nc.tensor.matmul(
                        psd,
                        lhsT=sT_all[:, kj, :],
                        rhs=wdt_all[:, kj, sl],
                        start=(kj == 0), stop=False,
                    )
                nc.tensor.matmul(
                    psd, lhsT=ones8, rhs=bd_b[0:1, sl],
                    start=False, stop=True,
                )
                nc.scalar.activation(dec_rows[:, sl], psd, AF.Copy)

            # EF stream buffers: partitions 119..126 = dec rows,
            # partition 127 = W_c (constant across batches/chunks)
            ef_bufs = [efp.tile([128, NCH, N], BF16, tag="ef", name=f"efb{i}")
                       for i in range(2)]
            for buf in ef_bufs:
                for c in range(NCH):
                    nc.sync.dma_start(out=buf[119:127, c, :], in_=dec_rows)
                    nc.sync.dma_start(out=buf[127:128, c, :], in_=wc_b)

            # ---------------- pass 1: scores ----------------
            def chunk_m(c):
                return LAST if c == NCH - 1 else CH

            score_cols_t = {}

            def phase_a(b):
                buf = ef_bufs[b % 2]
                if b == 0:
                    # per-pair loads so the first chunks land fast at start
                    for c2 in range(4):
                        nc.gpsimd.dma_start(
                            out=buf[0:CH, 2 * c2:2 * c2 + 2, :],
                            in_=efA_d[b, c2, :, :],
                        )
                else:
                    nc.gpsimd.dma_start(
                        out=buf[0:CH, 0:NCH - 1, :],
                        in_=efA_d[b, :, :, :].rearrange("c p m -> p c m"),
                    )
                nc.gpsimd.dma_start(
                    out=buf[0:CH, NCH - 1, :], in_=efB_d[b, :, :]
                )

                score_cols = smal.tile([CH, NCH], F32, tag="scol")
                score_cols_t[b] = score_cols
                for c in range(NCH):
                    m = chunk_m(c)
                    att = psA.tile([CH, N], F32, tag="att")
                    for h in range(2):
                        nc.tensor.matmul(
                            att[0:m, h * 512:(h + 1) * 512],
                            lhsT=lhs_all[:, b * W + c * CH: b * W + c * CH + m],
                            rhs=buf[:, c, h * 512:(h + 1) * 512],
                            start=True, stop=True, skip_group_check=True,
                        )
                    th = thp.tile([CH, N], BF16, tag="th")
                    nc.scalar.activation(th[0:m, :], att[0:m, :], AF.Tanh)
                    scr = ttro.tile([CH, N], BF16, tag="ttro")
                    nc.vector.scalar_tensor_tensor(
                        out=scr[0:m, :], in0=th[0:m, :], scalar=1.0,
                        in1=v_bcast[0:m, :],
                        op0=ALU.mult, op1=ALU.mult,
                        accum_out=score_cols[0:m, c:c + 1],
                    )

                # score columns -> row b of score_all (t = c*119 + p)
                ps9 = psT.tile([NCH, CH], F32, tag="tscratch")
                nc.tensor.matmul(
                    ps9, lhsT=score_cols, rhs=ident, is_transpose=True,
                    start=True, stop=True,
                )
                score9 = smal.tile([NCH, CH], F32, tag="s9")
                nc.scalar.activation(score9, ps9, AF.Copy)
                srow = score_g[b // GRP]
                r = b % GRP
                nc.sync.dma_start(
                    out=srow[r:r + 1, 0:(NCH - 1) * CH].rearrange(
                        "p (c w) -> p c w", c=NCH - 1),
                    in_=score9[0:NCH - 1, :],
                )
                nc.sync.dma_start(
                    out=srow[r:r + 1, (NCH - 1) * CH:T],
                    in_=score9[NCH - 1:NCH, 0:LAST],
                )

            # masked softmax + renorm + coverage for GRP batches at once
            def softmax_group(g):
                # scores are O(1) (|s| < ~3): plain exp is safe, skip max-sub
                nc.scalar.activation(attn_g[g], score_g[g], AF.Exp)
                ssum = smal.tile([GRP, 1], F32, tag="ssum")
                nc.vector.scalar_tensor_tensor(
                    out=attn_g[g], in0=attn_g[g], scalar=1.0,
                    in1=mask_g[g],
                    op0=ALU.mult, op1=ALU.mult, accum_out=ssum,
                )
                rs = smal.tile([GRP, 1], F32, tag="rs")
                nc.vector.reciprocal(rs, ssum)
                nc.vector.tensor_scalar_mul(attn_g[g], attn_g[g], rs)
                nc.vector.tensor_add(covn_g[g], cov_g[g], attn_g[g])

            # ---------------- pass 2: context vectors ----------------
            eo_bufs = {}

            def load_eo(b):
                buf = eop.tile([CH, NCH, N], BF16, tag="eo")
                nc.gpsimd.dma_start(
                    out=buf[:, 0:NCH - 1, :],
                    in_=eoA_d[b, :, :, :].rearrange("c p m -> p c m"),
                )
                nc.gpsimd.dma_start(
                    out=buf[:, NCH - 1, :], in_=eoB_d[b, :, :]
                )
                eo_bufs[b] = buf

            ctps = [psS.tile([NB, 512], F32, tag="srow", name=f"ctp{h}")
                    for h in range(2)]

            def phase_c(b):
                # attn row -> [9, 119] -> transpose -> one-hot column b
                attn9 = a9p.tile([NCH, CH], F32, tag="attn9")
                arow = attn_g[b // GRP]
                r = b % GRP
                nc.sync.dma_start(
                    out=attn9[0:NCH - 1, :],
                    in_=arow[r:r + 1, 0:(NCH - 1) * CH].rearrange(
                        "p (c w) -> p c w", c=NCH - 1),
                )
                nc.sync.dma_start(
                    out=attn9[NCH - 1:NCH, 0:LAST],
                    in_=arow[r:r + 1, (NCH - 1) * CH:T],
                )
                acp = psT.tile([CH, NCH], F32, tag="tscratch")
                nc.tensor.matmul(
                    acp, lhsT=attn9, rhs=ident[0:NCH, 0:NCH],
                    is_transpose=True, start=True, stop=True,
                )
                acw = acwp.tile([CH, NCH, NB], BF16, tag="acw")
                nc.gpsimd.memset(acw, 0.0)
                nc.scalar.activation(acw[:, :, b], acp, AF.Copy)

                buf = eo_bufs.pop(b)
                for c in range(NCH):
                    m = chunk_m(c)
                    for h in range(2):
                        nc.tensor.matmul(
                            ctps[h],
                            lhsT=acw[0:m, c, :],
                            rhs=buf[0:m, c, h * 512:(h + 1) * 512],
                            start=(b == 0 and c == 0),
                            stop=(b == nb - 1 and c == NCH - 1),
                            skip_group_check=True,
                        )

            # ---------------- schedule ----------------
            # pass 1 (scores), one softmax, pass 2 (context vectors);
            # EO prefetch limited to pool depth to avoid queue blocking
            for b in range(nb):
                phase_a(b)
                if 2 <= b <= 4:
                    load_eo(b - 2)
            softmax_group(0)
            for b in range(nb):
                phase_c(b)
                if b + 3 < nb:
                    load_eo(b + 3)

            for h in range(2):
                nc.scalar.activation(
                    ct_sb[:, h * 512:(h + 1) * 512], ctps[h], AF.Copy
                )
            nc.sync.dma_start(out=ct_o[:, :], in_=ct_sb)
            for g in range(ngrp):
                rows = slice(g * GRP, (g + 1) * GRP)
                nc.sync.dma_start(out=at_o[rows, :], in_=attn_g[g])
                nc.sync.dma_start(out=cn_o[rows, :], in_=covn_g[g])

    nc.finalize()
    return nc


_CACHE: dict = {}


def _get_nc() -> bass.Bass:
    if "nc" not in _CACHE:
        _CACHE["nc"] = build_bass(NB)
    return _CACHE["nc"]


def _stage_tiles(x):
    """[T, N] f32 -> ([4, 119, 2048], [119, 1024]) bf16.

    Chunk-pair-major layout: per-partition DMA descriptor runs are
    exactly 4KB and non-adjacent in DRAM (so they cannot be coalesced
    into slow single-partition mega-packets)."""
    pad = np.zeros((CH * NCH, N), np.float32)
    pad[:T] = x
    a = pad.reshape(NCH, CH, N)
    main = np.ascontiguousarray(
        a[0:NCH - 1].reshape(4, 2, CH, N).transpose(0, 2, 1, 3)
        .reshape(4, CH, 2 * N)
    ).astype(ml_dtypes.bfloat16)
    tail = a[NCH - 1].astype(ml_dtypes.bfloat16)
    return main, tail


def _build_lhs(cov_core):
    """identity band + one-hot rows + cov row, split into 4KB-line part A
    ([4, 128, 2048], cols 0..8191 pair-packed) and tail part B."""
    lhs = np.zeros((128, NB * W), np.float32)
    eye = np.eye(CH, dtype=np.float32)
    lhs[:CH] = np.tile(eye, (1, NB * NCH))
    for j in range(NB):
        lhs[CH + j, j * W:(j + 1) * W] = 1.0
        lhs[127, j * W:j * W + T] = cov_core[j]
    lhs = lhs.astype(ml_dtypes.bfloat16)
    a = np.ascontiguousarray(lhs[:, 0:8192].reshape(128, 4, 2048).transpose(1, 0, 2))
    b = np.ascontiguousarray(lhs[:, 8192:])
    return a, b


def make_in_maps(inputs: dict) -> list:
    f = lambda x: np.ascontiguousarray(np.asarray(x), dtype=np.float32)
    s = f(inputs["s_t_hat"])
    eo = f(inputs["encoder_outputs"])
    ef = f(inputs["encoder_feature"]).reshape(B, T, N)
    mk = f(inputs["enc_padding_mask"])
    cv = f(inputs["coverage"])
    wdt = np.ascontiguousarray(f(inputs["W_d"]).T).astype(ml_dtypes.bfloat16)
    # [1024, 1024] -> k-tile pair layout [4, 128, 2048] (4KB DMA lines):
    # wdt_all[p, kj, :] = W_d_T[kj*128 + p, :]
    wdt_pairs = np.ascontiguousarray(
        wdt.reshape(KT, 128, N).transpose(1, 0, 2).reshape(128, 4, 2048)
        .transpose(1, 0, 2)
    )
    bd = f(inputs["b_d"])
    wc = f(inputs["W_c"])
    vv = f(inputs["v"])
    in_maps = []
    for i in range(N_CORES):
        sl = slice(i * NB, (i + 1) * NB)
        ef_st = [_stage_tiles(ef[i * NB + j]) for j in range(NB)]
        eo_st = [_stage_tiles(eo[i * NB + j]) for j in range(NB)]
        lhs_a, lhs_b = _build_lhs(cv[sl])
        in_maps.append({
            "ef_a": np.stack([m for m, _ in ef_st]),
            "ef_b": np.stack([t for _, t in ef_st]),
            "eo_a": np.stack([m for m, _ in eo_st]),
            "eo_b": np.stack([t for _, t in eo_st]),
            "lhsT_a": lhs_a,
            "lhsT_b": lhs_b,
            "s_t_hat_T": np.ascontiguousarray(s[sl].T).astype(ml_dtypes.bfloat16),
            "enc_padding_mask": mk[sl],
            "coverage": cv[sl],
            "W_d_T": wdt_pairs,
            "b_d": bd.astype(ml_dtypes.bfloat16),
            "W_c": wc.astype(ml_dtypes.bfloat16),
            "v": vv.astype(ml_dtypes.bfloat16),
        })
    return in_maps


def gather_outputs(results: list):
    c_t = np.concatenate([results[i]["c_t"] for i in range(N_CORES)], axis=0)
    attn = np.concatenate([results[i]["attn"] for i in range(N_CORES)], axis=0)
    covn = np.concatenate(
        [results[i]["coverage_next"] for i in range(N_CORES)], axis=0
    )
    return c_t, attn, covn


def kernel(**inputs):
    nc = _get_nc()
    in_maps = make_in_maps(inputs)
    res = run_bass_kernel_spmd(nc, in_maps, core_ids=list(range(N_CORES)))
    return gather_outputs(res.results)
